# revision 29
# baseline (speedup 1.0000x reference)
"""nn_ApproximateEuclideanAttention — 8-way sharded Bass/Trainium2 kernel.

Sharding: data-parallel over batch (2) x tensor-parallel over head-groups
(16 heads -> 4 groups of 4), one shard per NeuronCore. Each core holds the
head-slice of the K/V projections, a folded Q-side score projection, its own
per-head 64x64 landmark-kernel inverses (host-precomputed), and a column
slice of Wo; the 4 head-shard partial outputs of a batch are summed on host.

Device math (per core, per head h; n=8192 seq, E=1024, d=64, k=64 landmarks):
  scores_Q^T = (Mq_h^T x^T)          Mq_h = Wq_h^T (2/tau) L_h^T   (folded)
  Phi_Q'     = exp(scores_Q + cq)    cq_j = (-|L_j|^2 + 2 bq_h.L_j)/tau
  [K|V]      = x [WkT_h | WvT_h]     (one fused GEMM)
  Phi_K'     = exp((2/tau) K L^T - |K_n|^2/tau)
  Zaug       = Phi_K'^T [V | 1]      (k x 65, PSUM-accumulated over n)
  U          = G2_h Zaug             G2_h = W^-1 diag(exp(-|L_j|^2/tau))
  out9       = Phi_Q' U              (n x 65): cols 0:64 unnorm, col 64 norm
  H          = out9[:, :64] / out9[:, 64]
  O_partial  = H Wo[:, sl]^T         (n x E, f32, host-reduced over 4 shards)
The per-row exp(-|Q_n|^2/tau) factor of Phi_Q cancels in the ratio; the
per-landmark exp(-|L_j|^2/tau) factor of Phi_K commutes into G2. All GEMMs
run in bf16 with f32 PSUM accumulation.
"""

import numpy as np

EMBED_DIM = 1024
NUM_HEADS = 16
HEAD_DIM = EMBED_DIM // NUM_HEADS
NUM_LANDMARKS = 64
REG = 1e-6
BATCH = 2
SEQ = 8192
HEAD_GROUPS = 4          # head-parallel degree (4 heads per core)
HL = NUM_HEADS // HEAD_GROUPS  # heads per core = 4
HD = HL * HEAD_DIM       # 256 projection cols per core
N_CORES = 8
NCHUNK = 16              # seq chunks of 512
CS = SEQ // NCHUNK       # 512
NE = EMBED_DIM // 128    # 8 contraction chunks

_BUILT = None  # cached compiled Bass module

# Debug/measurement knobs (used by the dev test harness; harmless defaults)
TRACE = False          # capture NTFF profile on the next device run
DEBUG_RAISE = False    # raise device errors instead of numpy fallback
LAST_EXEC_NS = None    # exec_time_ns of the last traced run
LAST_TRACE = None      # BassKernelResults of the last traced run


def _install_ntff_shim():
    """Register the axon NTFF profile hook if the image lacks antenv.axon_hooks."""
    import sys as _sys
    import types as _types
    try:
        from antenv import axon_hooks  # noqa: F401
        return
    except ImportError:
        pass
    from trn_agent_boot.trn_boot import _ntff_profile_via_ctypes
    hook = _ntff_profile_via_ctypes("/opt/axon/libaxon_pjrt.so")
    mod = _types.ModuleType("antenv.axon_hooks")
    mod.get_axon_ntff_profile_hook = lambda: hook
    mod.set_axon_ntff_profile_hook = lambda h: None
    _sys.modules["antenv.axon_hooks"] = mod
    import antenv
    antenv.axon_hooks = mod


def _sq_dists(X, L):
    Xn = np.sum(X * X, axis=-1, keepdims=True)
    Ln = np.sum(L * L, axis=-1, keepdims=True)
    XL = X @ L.T
    return np.maximum(Xn + Ln.T - 2.0 * XL, 0.0)


def _kernel_numpy(query, Wq, bq, Wk, bk, Wv, bv, Wo, bo, temperature, landmark_idx):
    """Reference-faithful numpy fallback (general shapes/inputs)."""
    f32 = np.float32
    x = np.asarray(query, dtype=f32)
    b, n, E = x.shape
    nh = Wq.shape[0] // HEAD_DIM if Wq.shape[0] % HEAD_DIM == 0 else NUM_HEADS
    hd = Wq.shape[0] // nh
    tau = float(np.asarray(temperature))
    idx = np.asarray(landmark_idx).astype(np.int64)
    k = idx.shape[0]
    out = np.empty((b, n, Wo.shape[0]), dtype=f32)
    for bi in range(b):
        Q = (x[bi] @ np.asarray(Wq, f32).T + bq).reshape(n, nh, hd).transpose(1, 0, 2)
        K = (x[bi] @ np.asarray(Wk, f32).T + bk).reshape(n, nh, hd).transpose(1, 0, 2)
        V = (x[bi] @ np.asarray(Wv, f32).T + bv).reshape(n, nh, hd).transpose(1, 0, 2)
        acc = np.empty((nh, n, hd), dtype=f32)
        for h in range(nh):
            L = K[h][idx]
            Phi_Q = np.exp(-_sq_dists(Q[h], L) / tau)
            C_K = _sq_dists(K[h], L)
            Phi_K = np.exp(-C_K / tau)
            W = np.exp(-C_K[idx] / tau) + REG * np.eye(k, dtype=f32)
            Z = Phi_K.T @ V[h]
            Y = np.linalg.solve(W, Z)
            out_un = Phi_Q @ Y
            norm = Phi_Q @ np.linalg.solve(W, Phi_K.sum(0)[:, None])
            acc[h] = out_un / np.maximum(norm, 1e-10)
        H = acc.transpose(1, 0, 2).reshape(n, nh * hd)
        out[bi] = H @ np.asarray(Wo, f32).T + bo
    return out.astype(np.asarray(query).dtype, copy=False)


def _build():
    """Build + bass-compile the per-core Tile program (once per process)."""
    global _BUILT
    if _BUILT is not None:
        return _BUILT
    import concourse.bass as bass
    import concourse.mybir as mybir
    import concourse.bacc as bacc
    import concourse.tile as tile
    from concourse import masks

    f32, bf16 = mybir.dt.float32, mybir.dt.bfloat16
    nc = bacc.Bacc(None, target_bir_lowering=False, debug=False)

    xt = nc.dram_tensor("xt", (EMBED_DIM, SEQ), bf16, kind="ExternalInput")
    wkv = nc.dram_tensor("wkv", (NE, 128, 2 * HD), bf16, kind="ExternalInput")
    mq = nc.dram_tensor("mq", (NE, 2, 128, 128), bf16, kind="ExternalInput")
    cq = nc.dram_tensor("cq", (128, 2), f32, kind="ExternalInput")
    a2 = nc.dram_tensor("a2", (128, 2, 64), bf16, kind="ExternalInput")
    gz = nc.dram_tensor("gz", (64, HL, 64), bf16, kind="ExternalInput")
    wo = nc.dram_tensor("wo", (2, 128, EMBED_DIM), bf16, kind="ExternalInput")
    op = nc.dram_tensor("op", (SEQ, EMBED_DIM), f32, kind="ExternalOutput")

    ntau = nc.dram_tensor("ntau", (128, 1), f32, kind="ExternalInput")

    MS = bass.MemorySpace
    Exp = mybir.ActivationFunctionType.Exp

    with tile.TileContext(nc) as tc:
        with tc.tile_pool(name="const", bufs=1) as cpool:
            ident = cpool.tile([128, 128], bf16)
            masks.make_identity(nc, ident[:])
            wkv_sb = cpool.tile([128, NE, 2 * HD], bf16)
            nc.sync.dma_start(wkv_sb[:], wkv[:].rearrange("e q f -> q e f"))
            mq_sb = cpool.tile([128, NE, 2, 128], bf16)
            nc.sync.dma_start(mq_sb[:], mq[:].rearrange("e p q b -> q e p b"))
            cq_sb = cpool.tile([128, 2], f32)
            nc.sync.dma_start(cq_sb[:], cq[:])
            a2_sb = cpool.tile([128, 2, 64], bf16)
            nc.sync.dma_start(a2_sb[:], a2[:])
            gz_sb = cpool.tile([64, HL, 64], bf16)
            nc.sync.dma_start(gz_sb[:], gz[:])
            wo_sb = cpool.tile([128, 2, EMBED_DIM], bf16)
            nc.sync.dma_start(wo_sb[:], wo[:].rearrange("c q f -> q c f"))

            phiqt0 = cpool.tile([128, SEQ], bf16, tag="phiqt0")
            phiqt1 = cpool.tile([128, SEQ], bf16, tag="phiqt1")
            phiqt = [phiqt0, phiqt1]
            zaug_sb = cpool.tile([64, HL, 65], bf16)
            u_tmp = cpool.tile([64, HL, 65], bf16)
            u_blk = cpool.tile([128, 2, 130], bf16)
            ntau_sb = cpool.tile([128, 1], f32, tag="ntau")  # -1/tau constant
            nc.sync.dma_start(ntau_sb[:], ntau[:])

            with (
                tc.tile_pool(name="xp", bufs=3) as xp,
                tc.tile_pool(name="sqp", bufs=2, space=MS.PSUM) as sqp,
                tc.tile_pool(name="kvp", bufs=1, space=MS.PSUM) as kvp,
                tc.tile_pool(name="ktp", bufs=1, space=MS.PSUM) as ktp,
                tc.tile_pool(name="skp", bufs=1, space=MS.PSUM) as skp,
                tc.tile_pool(name="zp", bufs=1, space=MS.PSUM) as zp,
                tc.tile_pool(name="wk1", bufs=3) as wk1,
            ):
                zaug = zp.tile([64, HL, 65], f32)
                for c in range(NCHUNK):
                    xt_sb = xp.tile([128, NE, CS], bf16)
                    nc.sync.dma_start(
                        xt_sb[:],
                        xt[:, c * CS:(c + 1) * CS].rearrange("(e q) s -> q e s", q=128),
                    )
                    kv_tiles = []
                    for m in range(4):
                        kv_ps = kvp.tile([128, 2 * HD], f32, tag=f"kv{m % 2}")
                        kv_tiles.append(kv_ps)
                        for e in range(NE):
                            nc.tensor.matmul(
                                kv_ps[:], xt_sb[:, e, m * 128:(m + 1) * 128],
                                wkv_sb[:, e, :],
                                start=(e == 0), stop=(e == NE - 1),
                            )
                    for p in range(2):
                        sq_ps = sqp.tile([128, CS], f32)
                        for e in range(NE):
                            nc.tensor.matmul(
                                sq_ps[:], mq_sb[:, e, p, :], xt_sb[:, e, :],
                                start=(e == 0), stop=(e == NE - 1),
                            )
                        nc.scalar.activation(
                            phiqt[p][:, c * CS:(c + 1) * CS], sq_ps[:],
                            Exp, bias=cq_sb[:, p:p + 1],
                        )
                    preps = []
                    for m in range(4):
                        kv_ps = kv_tiles[m]
                        k_sb = wk1.tile([128, HD], bf16, tag=f"k_sb{m}")
                        nc.vector.tensor_copy(k_sb[:], kv_ps[:, 0:HD])
                        # fk = exp(-|K_n|^2/tau) per head (row factor of Phi_K)
                        ksq = wk1.tile([128, HD], f32, tag="ksq")
                        nc.vector.tensor_tensor(
                            out=ksq[:], in0=k_sb[:], in1=k_sb[:],
                            op=mybir.AluOpType.mult,
                        )
                        nksum = wk1.tile([128, HL], f32, tag="nksum")
                        nc.vector.reduce_sum(
                            nksum[:], ksq[:].rearrange("q (h d) -> q h d", h=HL),
                            axis=mybir.AxisListType.X,
                        )
                        fk = wk1.tile([128, HL], f32, tag="fk")
                        nc.scalar.activation(fk[:], nksum[:], Exp,
                                             scale=ntau_sb[:, 0:1])
                        # vaug = fk * [V | 1] (row factor folded into V side)
                        vaug = wk1.tile([128, HL, 65], bf16, tag=f"vaug{m}")
                        nc.vector.tensor_tensor(
                            out=vaug[:, :, 0:64],
                            in0=kv_ps[:, HD:2 * HD].rearrange(
                                "q (h d) -> q h d", h=HL),
                            in1=fk[:].to_broadcast((128, HL, 64)),
                            op=mybir.AluOpType.mult,
                        )
                        nc.vector.tensor_copy(
                            vaug[:, :, 64:65],
                            fk[:].rearrange("q (h o) -> q h o", o=1),
                        )
                        preps.append((k_sb, vaug))
                    esks_all = []
                    for m in range(4):
                        k_sb, vaug = preps[m]
                        # K^T via pair transposes; S_K in two fixed-base banks
                        kt_ps = ktp.tile([128, 2, 128], bf16)
                        for cc in range(2):
                            nc.tensor.matmul(
                                kt_ps[:, cc, :], k_sb[:, cc * 128:(cc + 1) * 128],
                                ident[:], is_transpose=True,
                                start=(cc == 0), stop=(cc == 1),
                            )
                        kt_sb = wk1.tile([128, 2, 128], bf16, tag="kt_sb")
                        nc.vector.tensor_copy(kt_sb[:], kt_ps[:])
                        ska = skp.tile([128, 2, 64], f32, tag="ska")
                        skb = skp.tile([128, 2, 64], f32, tag="skb")
                        for j in range(2):      # heads 0,2 at base 0
                            nc.tensor.matmul(
                                ska[:, j, :], kt_sb[0:64, j, :],
                                a2_sb[0:64, j, :],
                                start=(j == 0), stop=(j == 1),
                            )
                        for j in range(2):      # heads 1,3 at base 64
                            nc.tensor.matmul(
                                skb[:, j, :], kt_sb[64:128, j, :],
                                a2_sb[64:128, j, :],
                                start=(j == 0), stop=(j == 1),
                            )
                        eska = wk1.tile([128, 2, 64], bf16, tag=f"eska{m}")
                        nc.scalar.activation(eska[:], ska[:], Exp)
                        eskb = wk1.tile([128, 2, 64], bf16, tag=f"eskb{m}")
                        nc.scalar.activation(eskb[:], skb[:], Exp)
                        esks_all.append((eska, eskb))
                    for m in range(4):
                        eska, eskb = esks_all[m]
                        vaug = preps[m][1]
                        esks = [eska, eskb]
                        for h in (0, 2, 1, 3):
                            nc.tensor.matmul(
                                zaug[:, h, :],
                                esks[h % 2][:, h // 2, :],
                                vaug[:, h, :],
                                start=(c == 0 and m == 0 and h == 0),
                                stop=(c == NCHUNK - 1 and m == 3 and h == 3),
                            )
                # U = G2 @ Zaug (per head), then block-diagonal pair layout
                nc.vector.tensor_copy(zaug_sb[:], zaug[:])
                u_ps = zp.tile([64, HL, 65], f32, tag="zaug")
                for h in range(HL):
                    nc.tensor.matmul(
                        u_ps[:, h, :], gz_sb[:, h, :], zaug_sb[:, h, :],
                        start=(h == 0), stop=(h == HL - 1),
                    )
                nc.vector.tensor_copy(u_tmp[:], u_ps[:])
                nc.vector.memset(u_blk[:], 0.0)
                for h in range(HL):
                    o = (h % 2) * 64
                    nc.sync.dma_start(
                        u_blk[o:o + 64, h // 2, (h % 2) * 65:(h % 2) * 65 + 65],
                        u_tmp[:, h, :])

            with (
                tc.tile_pool(name="o9p", bufs=2, space=MS.PSUM) as o9p,
                tc.tile_pool(name="htp", bufs=2, space=MS.PSUM) as htp,
                tc.tile_pool(name="oop", bufs=2, space=MS.PSUM) as oop,
                tc.tile_pool(name="wk2", bufs=3) as wk2,
            ):
                for c in range(NCHUNK):
                    hs = []
                    for m in range(4):
                        s0 = c * CS + m * 128
                        o9 = o9p.tile([128, 2, 130], f32)
                        for p in range(2):
                            nc.tensor.matmul(
                                o9[:, p, :], phiqt[p][:, s0:s0 + 128],
                                u_blk[:, p, :],
                                start=(p == 0), stop=(p == 1),
                            )
                        rec = wk2.tile([128, 2, 2], f32, tag="rec")
                        nc.vector.reciprocal(
                            rec[:],
                            o9[:].rearrange("q p (j x) -> q p j x", j=2)[:, :, :, 64],
                        )
                        h_sb = wk2.tile([128, HL, 64], bf16, tag=f"h_sb{m}")
                        nc.vector.tensor_tensor(
                            out=h_sb[:].rearrange("q (p j) d -> q p j d", p=2),
                            in0=o9[:].rearrange("q p (j x) -> q p j x", j=2)[:, :, :, 0:64],
                            in1=rec[:].to_broadcast((128, 2, 2, 64)),
                            op=mybir.AluOpType.mult,
                        )
                        hs.append(h_sb)
                    for m in range(4):
                        s0 = c * CS + m * 128
                        h_sb = hs[m]
                        ht_ps = htp.tile([128, 2, 128], bf16)
                        hflat = h_sb[:].rearrange("q h d -> q (h d)")
                        for cc in range(2):
                            nc.tensor.matmul(
                                ht_ps[:, cc, :], hflat[:, cc * 128:(cc + 1) * 128],
                                ident[:], is_transpose=True,
                                start=(cc == 0), stop=(cc == 1),
                            )
                        ht_sb = wk2.tile([128, 2, 128], bf16, tag="ht_sb")
                        if m % 2 == 0:
                            nc.vector.tensor_copy(ht_sb[:], ht_ps[:])
                        else:
                            nc.scalar.copy(ht_sb[:], ht_ps[:])
                        o_ps = oop.tile([128, EMBED_DIM], f32)
                        for cc in range(2):
                            for nn in range(2):
                                nc.tensor.matmul(
                                    o_ps[:, nn * 512:(nn + 1) * 512],
                                    ht_sb[:, cc, :],
                                    wo_sb[:, cc, nn * 512:(nn + 1) * 512],
                                    start=(cc == 0), stop=(cc == 1),
                                )
                        o_sb = wk2.tile([128, EMBED_DIM], f32, tag="o_sb")
                        if m % 2 == 0:
                            nc.vector.tensor_copy(o_sb[:], o_ps[:])
                        else:
                            nc.scalar.copy(o_sb[:], o_ps[:])
                        nc.sync.dma_start(op[s0:s0 + 128, :], o_sb[:])

    nc.compile()
    _BUILT = nc
    return nc


def _prep_core_inputs(x_b, xt_b, Wq, bq, Wk, bk, Wv, bv, tau, idx, g):
    """Host marshalling for core (batch b, head-group g)."""
    import ml_dtypes
    bf16 = ml_dtypes.bfloat16
    f32 = np.float32
    sl = slice(g * HD, (g + 1) * HD)
    WkT = np.ascontiguousarray(Wk[sl].T)  # (E, 256)
    WvT = np.ascontiguousarray(Wv[sl].T)
    wkv = np.concatenate([WkT, WvT], axis=1).reshape(NE, 128, 2 * HD)

    # landmarks from K rows (exact, includes bk)
    K_land = x_b[idx] @ Wk.T + bk            # (64, E) f32
    mq = np.empty((EMBED_DIM, HD), dtype=f32)
    a2 = np.empty((128, 2, 64), dtype=f32)
    gzm = np.empty((64, HL, 64), dtype=f32)
    cqv = np.empty((HD,), dtype=f32)
    for hl in range(HL):
        h = g * HL + hl
        hs = slice(h * HEAD_DIM, (h + 1) * HEAD_DIM)
        L = K_land[:, hs].astype(np.float64)          # (64, 64)
        A2 = (2.0 / tau) * L.T                        # (d, k)
        a2[(hl % 2) * 64:(hl % 2) * 64 + 64, hl // 2, :] = A2.astype(f32)
        Ln2 = np.sum(L * L, axis=1)                   # |L_j|^2
        bq_h = bq[hs].astype(np.float64)
        cqv[hl * 64:(hl + 1) * 64] = ((-Ln2 + 2.0 * (bq_h @ L.T)) / tau).astype(f32)
        Wq_h = Wq[hs].astype(np.float64)              # (64, E)
        mq[:, hl * 64:(hl + 1) * 64] = (Wq_h.T @ A2).astype(f32)
        # landmark kernel + inverse
        D = np.maximum(Ln2[:, None] + Ln2[None, :] - 2.0 * (L @ L.T), 0.0)
        Wk_mat = np.exp(-D / tau) + REG * np.eye(NUM_LANDMARKS)
        Winv = np.linalg.inv(Wk_mat)
        gzm[:, hl, :] = (np.exp(-Ln2 / tau)[:, None] * Winv).astype(f32)  # G2^T
    # lhsT pair tiles: (e, p, 128, 128); pair p covers local heads 2p,2p+1
    mq_t = mq.reshape(NE, 128, 2, 128).transpose(0, 2, 1, 3)
    cq_t = cqv.reshape(2, 128).T.copy()              # (128, 2)
    return {
        "ntau": np.full((128, 1), -1.0 / tau, dtype=f32),
        "a2": a2.astype(bf16),
        "xt": xt_b,
        "wkv": np.ascontiguousarray(wkv).astype(bf16),
        "mq": np.ascontiguousarray(mq_t).astype(bf16),
        "cq": cq_t.astype(f32),
        "gz": gzm.astype(bf16),
    }


def kernel(query, Wq, bq, Wk, bk, Wv, bv, Wo, bo, temperature, landmark_idx):
    query = np.asarray(query)
    out_dtype = query.dtype
    f32 = np.float32
    x = query.astype(f32, copy=False)
    Wq, Wk, Wv, Wo = (np.asarray(w).astype(f32, copy=False) for w in (Wq, Wk, Wv, Wo))
    bq, bk, bv, bo = (np.asarray(v).astype(f32, copy=False) for v in (bq, bk, bv, bo))
    tau = float(np.asarray(temperature))
    idx = np.asarray(landmark_idx).astype(np.int64)

    general_ok = (
        x.shape == (BATCH, SEQ, EMBED_DIM)
        and Wq.shape == (EMBED_DIM, EMBED_DIM)
        and idx.shape == (NUM_LANDMARKS,)
        and not np.any(bk)
        and not np.any(bv)
        and tau > 0
    )
    if not general_ok:
        return _kernel_numpy(query, Wq, bq, Wk, bk, Wv, bv, Wo, bo,
                             temperature, landmark_idx)
    try:
        return _kernel_device(x, Wq, bq, Wk, bk, Wv, bv, Wo, bo, tau, idx,
                              out_dtype)
    except Exception:
        if DEBUG_RAISE:
            raise
        return _kernel_numpy(query, Wq, bq, Wk, bk, Wv, bv, Wo, bo,
                             temperature, landmark_idx)


def _kernel_device(x, Wq, bq, Wk, bk, Wv, bv, Wo, bo, tau, idx, out_dtype):
    import ml_dtypes
    from concourse import bass_utils
    bf16 = ml_dtypes.bfloat16

    nc = _build()

    f32 = np.float32
    xt = [np.ascontiguousarray(x[b].T).astype(bf16) for b in range(BATCH)]
    wos = []
    for g in range(HEAD_GROUPS):
        sl = slice(g * HD, (g + 1) * HD)
        WoT = np.ascontiguousarray(Wo[:, sl].T)      # (256, E)
        wos.append(np.ascontiguousarray(WoT.reshape(2, 128, EMBED_DIM)).astype(bf16))

    in_maps = []
    for b in range(BATCH):
        for g in range(HEAD_GROUPS):
            m = _prep_core_inputs(x[b], xt[b], Wq, bq, Wk, bk, Wv, bv, tau, idx, g)
            m["wo"] = wos[g]
            in_maps.append(m)

    global LAST_EXEC_NS, LAST_TRACE
    kw = {}
    if TRACE:
        _install_ntff_shim()
        kw = {"trace": True, "trace_cores": [0]}
    res = bass_utils.run_bass_kernel_spmd(nc, in_maps,
                                          core_ids=list(range(N_CORES)), **kw)
    if TRACE:
        LAST_EXEC_NS = res.exec_time_ns
        LAST_TRACE = res
    out = np.zeros((BATCH, SEQ, EMBED_DIM), dtype=f32)
    for b in range(BATCH):
        for g in range(HEAD_GROUPS):
            out[b] += res.results[b * HEAD_GROUPS + g]["op"]
        out[b] += bo
    return out.astype(out_dtype, copy=False)


# revision 32
# speedup vs baseline: 1.0235x; 1.0235x over previous
"""nn_ApproximateEuclideanAttention — 8-way sharded Bass/Trainium2 kernel.

Sharding: data-parallel over batch (2) x tensor-parallel over head-groups
(16 heads -> 4 groups of 4), one shard per NeuronCore. Each core holds the
head-slice of the K/V projections, a folded Q-side score projection, its own
per-head 64x64 landmark-kernel inverses (host-precomputed), and a column
slice of Wo; the 4 head-shard partial outputs of a batch are summed on host.

Device math (per core, per head h; n=8192 seq, E=1024, d=64, k=64 landmarks):
  scores_Q^T = (Mq_h^T x^T)          Mq_h = Wq_h^T (2/tau) L_h^T   (folded)
  Phi_Q'     = exp(scores_Q + cq)    cq_j = (-|L_j|^2 + 2 bq_h.L_j)/tau
  [K|V]      = x [WkT_h | WvT_h]     (one fused GEMM)
  Phi_K'     = exp((2/tau) K L^T - |K_n|^2/tau)
  Zaug       = Phi_K'^T [V | 1]      (k x 65, PSUM-accumulated over n)
  U          = G2_h Zaug             G2_h = W^-1 diag(exp(-|L_j|^2/tau))
  out9       = Phi_Q' U              (n x 65): cols 0:64 unnorm, col 64 norm
  H          = out9[:, :64] / out9[:, 64]
  O_partial  = H Wo[:, sl]^T         (n x E, f32, host-reduced over 4 shards)
The per-row exp(-|Q_n|^2/tau) factor of Phi_Q cancels in the ratio; the
per-landmark exp(-|L_j|^2/tau) factor of Phi_K commutes into G2. All GEMMs
run in bf16 with f32 PSUM accumulation.
"""

import numpy as np

EMBED_DIM = 1024
NUM_HEADS = 16
HEAD_DIM = EMBED_DIM // NUM_HEADS
NUM_LANDMARKS = 64
REG = 1e-6
BATCH = 2
SEQ = 8192
HEAD_GROUPS = 4          # head-parallel degree (4 heads per core)
HL = NUM_HEADS // HEAD_GROUPS  # heads per core = 4
HD = HL * HEAD_DIM       # 256 projection cols per core
N_CORES = 8
NCHUNK = 16              # seq chunks of 512
CS = SEQ // NCHUNK       # 512
NE = EMBED_DIM // 128    # 8 contraction chunks

_BUILT = None  # cached compiled Bass module

# Debug/measurement knobs (used by the dev test harness; harmless defaults)
TRACE = False          # capture NTFF profile on the next device run
DEBUG_RAISE = False    # raise device errors instead of numpy fallback
LAST_EXEC_NS = None    # exec_time_ns of the last traced run
LAST_TRACE = None      # BassKernelResults of the last traced run


def _install_ntff_shim():
    """Register the axon NTFF profile hook if the image lacks antenv.axon_hooks."""
    import sys as _sys
    import types as _types
    try:
        from antenv import axon_hooks  # noqa: F401
        return
    except ImportError:
        pass
    from trn_agent_boot.trn_boot import _ntff_profile_via_ctypes
    hook = _ntff_profile_via_ctypes("/opt/axon/libaxon_pjrt.so")
    mod = _types.ModuleType("antenv.axon_hooks")
    mod.get_axon_ntff_profile_hook = lambda: hook
    mod.set_axon_ntff_profile_hook = lambda h: None
    _sys.modules["antenv.axon_hooks"] = mod
    import antenv
    antenv.axon_hooks = mod


def _sq_dists(X, L):
    Xn = np.sum(X * X, axis=-1, keepdims=True)
    Ln = np.sum(L * L, axis=-1, keepdims=True)
    XL = X @ L.T
    return np.maximum(Xn + Ln.T - 2.0 * XL, 0.0)


def _kernel_numpy(query, Wq, bq, Wk, bk, Wv, bv, Wo, bo, temperature, landmark_idx):
    """Reference-faithful numpy fallback (general shapes/inputs)."""
    f32 = np.float32
    x = np.asarray(query, dtype=f32)
    b, n, E = x.shape
    nh = Wq.shape[0] // HEAD_DIM if Wq.shape[0] % HEAD_DIM == 0 else NUM_HEADS
    hd = Wq.shape[0] // nh
    tau = float(np.asarray(temperature))
    idx = np.asarray(landmark_idx).astype(np.int64)
    k = idx.shape[0]
    out = np.empty((b, n, Wo.shape[0]), dtype=f32)
    for bi in range(b):
        Q = (x[bi] @ np.asarray(Wq, f32).T + bq).reshape(n, nh, hd).transpose(1, 0, 2)
        K = (x[bi] @ np.asarray(Wk, f32).T + bk).reshape(n, nh, hd).transpose(1, 0, 2)
        V = (x[bi] @ np.asarray(Wv, f32).T + bv).reshape(n, nh, hd).transpose(1, 0, 2)
        acc = np.empty((nh, n, hd), dtype=f32)
        for h in range(nh):
            L = K[h][idx]
            Phi_Q = np.exp(-_sq_dists(Q[h], L) / tau)
            C_K = _sq_dists(K[h], L)
            Phi_K = np.exp(-C_K / tau)
            W = np.exp(-C_K[idx] / tau) + REG * np.eye(k, dtype=f32)
            Z = Phi_K.T @ V[h]
            Y = np.linalg.solve(W, Z)
            out_un = Phi_Q @ Y
            norm = Phi_Q @ np.linalg.solve(W, Phi_K.sum(0)[:, None])
            acc[h] = out_un / np.maximum(norm, 1e-10)
        H = acc.transpose(1, 0, 2).reshape(n, nh * hd)
        out[bi] = H @ np.asarray(Wo, f32).T + bo
    return out.astype(np.asarray(query).dtype, copy=False)


def _build():
    """Build + bass-compile the per-core Tile program (once per process)."""
    global _BUILT
    if _BUILT is not None:
        return _BUILT
    import concourse.bass as bass
    import concourse.mybir as mybir
    import concourse.bacc as bacc
    import concourse.tile as tile
    from concourse import masks

    f32, bf16 = mybir.dt.float32, mybir.dt.bfloat16
    nc = bacc.Bacc(None, target_bir_lowering=False, debug=False)

    xt = nc.dram_tensor("xt", (EMBED_DIM, SEQ), bf16, kind="ExternalInput")
    wkv = nc.dram_tensor("wkv", (NE, 128, 2 * HD), bf16, kind="ExternalInput")
    mq = nc.dram_tensor("mq", (NE, 2, 128, 128), bf16, kind="ExternalInput")
    cq = nc.dram_tensor("cq", (128, 2), f32, kind="ExternalInput")
    a2 = nc.dram_tensor("a2", (128, 2, 64), bf16, kind="ExternalInput")
    gz = nc.dram_tensor("gz", (64, HL, 64), bf16, kind="ExternalInput")
    wo = nc.dram_tensor("wo", (2, 128, EMBED_DIM), bf16, kind="ExternalInput")
    op = nc.dram_tensor("op", (SEQ, EMBED_DIM), f32, kind="ExternalOutput")

    ntau = nc.dram_tensor("ntau", (128, 1), f32, kind="ExternalInput")

    MS = bass.MemorySpace
    Exp = mybir.ActivationFunctionType.Exp

    with tile.TileContext(nc) as tc:
        with tc.tile_pool(name="const", bufs=1) as cpool:
            ident = cpool.tile([128, 128], bf16)
            masks.make_identity(nc, ident[:])
            wkv_sb = cpool.tile([128, NE, 2 * HD], bf16)
            nc.sync.dma_start(wkv_sb[:], wkv[:].rearrange("e q f -> q e f"))
            mq_sb = cpool.tile([128, NE, 2, 128], bf16)
            nc.sync.dma_start(mq_sb[:], mq[:].rearrange("e p q b -> q e p b"))
            cq_sb = cpool.tile([128, 2], f32)
            nc.sync.dma_start(cq_sb[:], cq[:])
            a2_sb = cpool.tile([128, 2, 64], bf16)
            nc.sync.dma_start(a2_sb[:], a2[:])
            gz_sb = cpool.tile([64, HL, 64], bf16)
            wo_sb = cpool.tile([128, 2, EMBED_DIM], bf16)

            phiqt0 = cpool.tile([128, SEQ], bf16, tag="phiqt0")
            phiqt1 = cpool.tile([128, SEQ], bf16, tag="phiqt1")
            phiqt = [phiqt0, phiqt1]
            zaug_sb = cpool.tile([64, HL, 65], bf16)
            u_tmp = cpool.tile([64, HL, 65], bf16)
            u_blk = cpool.tile([128, 2, 130], bf16)
            ntau_sb = cpool.tile([128, 1], f32, tag="ntau")  # -1/tau constant
            nc.sync.dma_start(ntau_sb[:], ntau[:])

            with (
                tc.tile_pool(name="xp", bufs=4) as xp,
                tc.tile_pool(name="sqp", bufs=2, space=MS.PSUM) as sqp,
                tc.tile_pool(name="kvp", bufs=1, space=MS.PSUM) as kvp,
                tc.tile_pool(name="ktp", bufs=1, space=MS.PSUM) as ktp,
                tc.tile_pool(name="skp", bufs=1, space=MS.PSUM) as skp,
                tc.tile_pool(name="zp", bufs=1, space=MS.PSUM) as zp,
                tc.tile_pool(name="wk1", bufs=3) as wk1,
            ):
                zaug = zp.tile([64, HL, 65], f32)
                for c in range(NCHUNK):
                    xt_sb = xp.tile([128, NE, CS], bf16)
                    nc.sync.dma_start(
                        xt_sb[:],
                        xt[:, c * CS:(c + 1) * CS].rearrange("(e q) s -> q e s", q=128),
                    )
                    if c == 1:
                        nc.sync.dma_start(gz_sb[:], gz[:])
                        nc.sync.dma_start(wo_sb[:],
                                          wo[:].rearrange("c q f -> q c f"))
                    kv_tiles = []
                    for m in range(4):
                        kv_ps = kvp.tile([128, 2 * HD], f32, tag=f"kv{m % 2}")
                        kv_tiles.append(kv_ps)
                        for e in range(NE):
                            nc.tensor.matmul(
                                kv_ps[:], xt_sb[:, e, m * 128:(m + 1) * 128],
                                wkv_sb[:, e, :],
                                start=(e == 0), stop=(e == NE - 1),
                            )
                    for p in range(2):
                        sq_ps = sqp.tile([128, CS], f32)
                        for e in range(NE):
                            nc.tensor.matmul(
                                sq_ps[:], mq_sb[:, e, p, :], xt_sb[:, e, :],
                                start=(e == 0), stop=(e == NE - 1),
                            )
                        nc.scalar.activation(
                            phiqt[p][:, c * CS:(c + 1) * CS], sq_ps[:],
                            Exp, bias=cq_sb[:, p:p + 1],
                        )
                    preps = []
                    for m in range(4):
                        kv_ps = kv_tiles[m]
                        k_sb = wk1.tile([128, HD], bf16, tag=f"k_sb{m}")
                        nc.vector.tensor_copy(k_sb[:], kv_ps[:, 0:HD])
                        # fk = exp(-|K_n|^2/tau) per head (row factor of Phi_K)
                        ksq = wk1.tile([128, HD], f32, tag="ksq")
                        nc.vector.tensor_tensor(
                            out=ksq[:], in0=k_sb[:], in1=k_sb[:],
                            op=mybir.AluOpType.mult,
                        )
                        nksum = wk1.tile([128, HL], f32, tag="nksum")
                        nc.vector.reduce_sum(
                            nksum[:], ksq[:].rearrange("q (h d) -> q h d", h=HL),
                            axis=mybir.AxisListType.X,
                        )
                        fk = wk1.tile([128, HL], f32, tag="fk")
                        nc.scalar.activation(fk[:], nksum[:], Exp,
                                             scale=ntau_sb[:, 0:1])
                        # vaug = fk * [V | 1] (row factor folded into V side)
                        vaug = wk1.tile([128, HL, 65], bf16, tag=f"vaug{m}")
                        nc.vector.tensor_tensor(
                            out=vaug[:, :, 0:64],
                            in0=kv_ps[:, HD:2 * HD].rearrange(
                                "q (h d) -> q h d", h=HL),
                            in1=fk[:].to_broadcast((128, HL, 64)),
                            op=mybir.AluOpType.mult,
                        )
                        nc.vector.tensor_copy(
                            vaug[:, :, 64:65],
                            fk[:].rearrange("q (h o) -> q h o", o=1),
                        )
                        preps.append((k_sb, vaug))
                    esks_all = []
                    for m in range(4):
                        k_sb, vaug = preps[m]
                        # K^T via pair transposes; S_K in two fixed-base banks
                        kt_ps = ktp.tile([128, 2, 128], bf16)
                        for cc in range(2):
                            nc.tensor.matmul(
                                kt_ps[:, cc, :], k_sb[:, cc * 128:(cc + 1) * 128],
                                ident[:], is_transpose=True,
                                start=(cc == 0), stop=(cc == 1),
                            )
                        kt_sb = wk1.tile([128, 2, 128], bf16, tag="kt_sb")
                        nc.vector.tensor_copy(kt_sb[:], kt_ps[:])
                        ska = skp.tile([128, 2, 64], f32, tag="ska")
                        skb = skp.tile([128, 2, 64], f32, tag="skb")
                        for j in range(2):      # heads 0,2 at base 0
                            nc.tensor.matmul(
                                ska[:, j, :], kt_sb[0:64, j, :],
                                a2_sb[0:64, j, :],
                                start=(j == 0), stop=(j == 1),
                            )
                        for j in range(2):      # heads 1,3 at base 64
                            nc.tensor.matmul(
                                skb[:, j, :], kt_sb[64:128, j, :],
                                a2_sb[64:128, j, :],
                                start=(j == 0), stop=(j == 1),
                            )
                        eska = wk1.tile([128, 2, 64], bf16, tag=f"eska{m}")
                        nc.scalar.activation(eska[:], ska[:], Exp)
                        eskb = wk1.tile([128, 2, 64], bf16, tag=f"eskb{m}")
                        nc.scalar.activation(eskb[:], skb[:], Exp)
                        esks_all.append((eska, eskb))
                    for m in range(4):
                        eska, eskb = esks_all[m]
                        vaug = preps[m][1]
                        esks = [eska, eskb]
                        for h in (0, 2, 1, 3):
                            nc.tensor.matmul(
                                zaug[:, h, :],
                                esks[h % 2][:, h // 2, :],
                                vaug[:, h, :],
                                start=(c == 0 and m == 0 and h == 0),
                                stop=(c == NCHUNK - 1 and m == 3 and h == 3),
                            )
                # U = G2 @ Zaug (per head), then block-diagonal pair layout
                nc.vector.tensor_copy(zaug_sb[:], zaug[:])
                u_ps = zp.tile([64, HL, 65], f32, tag="zaug")
                for h in range(HL):
                    nc.tensor.matmul(
                        u_ps[:, h, :], gz_sb[:, h, :], zaug_sb[:, h, :],
                        start=(h == 0), stop=(h == HL - 1),
                    )
                nc.vector.tensor_copy(u_tmp[:], u_ps[:])
                nc.vector.memset(u_blk[:], 0.0)
                for h in range(HL):
                    o = (h % 2) * 64
                    nc.sync.dma_start(
                        u_blk[o:o + 64, h // 2, (h % 2) * 65:(h % 2) * 65 + 65],
                        u_tmp[:, h, :])

            with (
                tc.tile_pool(name="o9p", bufs=2, space=MS.PSUM) as o9p,
                tc.tile_pool(name="htp", bufs=2, space=MS.PSUM) as htp,
                tc.tile_pool(name="oop", bufs=2, space=MS.PSUM) as oop,
                tc.tile_pool(name="wk2", bufs=3) as wk2,
            ):
                for c in range(NCHUNK):
                    hs = []
                    for m in range(4):
                        s0 = c * CS + m * 128
                        o9 = o9p.tile([128, 2, 130], f32)
                        for p in range(2):
                            nc.tensor.matmul(
                                o9[:, p, :], phiqt[p][:, s0:s0 + 128],
                                u_blk[:, p, :],
                                start=(p == 0), stop=(p == 1),
                            )
                        rec = wk2.tile([128, 2, 2], f32, tag="rec")
                        nc.vector.reciprocal(
                            rec[:],
                            o9[:].rearrange("q p (j x) -> q p j x", j=2)[:, :, :, 64],
                        )
                        h_sb = wk2.tile([128, HL, 64], bf16, tag=f"h_sb{m}")
                        nc.vector.tensor_tensor(
                            out=h_sb[:].rearrange("q (p j) d -> q p j d", p=2),
                            in0=o9[:].rearrange("q p (j x) -> q p j x", j=2)[:, :, :, 0:64],
                            in1=rec[:].to_broadcast((128, 2, 2, 64)),
                            op=mybir.AluOpType.mult,
                        )
                        hs.append(h_sb)
                    for m in range(4):
                        s0 = c * CS + m * 128
                        h_sb = hs[m]
                        ht_ps = htp.tile([128, 2, 128], bf16)
                        hflat = h_sb[:].rearrange("q h d -> q (h d)")
                        for cc in range(2):
                            nc.tensor.matmul(
                                ht_ps[:, cc, :], hflat[:, cc * 128:(cc + 1) * 128],
                                ident[:], is_transpose=True,
                                start=(cc == 0), stop=(cc == 1),
                            )
                        ht_sb = wk2.tile([128, 2, 128], bf16, tag="ht_sb")
                        if m % 2 == 0:
                            nc.vector.tensor_copy(ht_sb[:], ht_ps[:])
                        else:
                            nc.scalar.copy(ht_sb[:], ht_ps[:])
                        o_ps0 = oop.tile([128, 512], f32, tag="ops0")
                        o_ps1 = oop.tile([128, 512], f32, tag="ops1")
                        o_tiles = [o_ps0, o_ps1]
                        for cc in range(2):
                            for nn in range(2):
                                nc.tensor.matmul(
                                    o_tiles[nn][:],
                                    ht_sb[:, cc, :],
                                    wo_sb[:, cc, nn * 512:(nn + 1) * 512],
                                    start=(cc == 0), stop=(cc == 1),
                                )
                        o_sb = wk2.tile([128, EMBED_DIM], f32, tag="o_sb")
                        nc.vector.tensor_copy(o_sb[:, 0:512], o_ps0[:])
                        nc.scalar.copy(o_sb[:, 512:1024], o_ps1[:])
                        nc.sync.dma_start(op[s0:s0 + 128, :], o_sb[:])

    nc.compile()
    _BUILT = nc
    return nc


def _prep_core_inputs(x_b, xt_b, Wq, bq, Wk, bk, Wv, bv, tau, idx, g):
    """Host marshalling for core (batch b, head-group g)."""
    import ml_dtypes
    bf16 = ml_dtypes.bfloat16
    f32 = np.float32
    sl = slice(g * HD, (g + 1) * HD)
    WkT = np.ascontiguousarray(Wk[sl].T)  # (E, 256)
    WvT = np.ascontiguousarray(Wv[sl].T)
    wkv = np.concatenate([WkT, WvT], axis=1).reshape(NE, 128, 2 * HD)

    # landmarks from K rows (exact, includes bk)
    K_land = x_b[idx] @ Wk.T + bk            # (64, E) f32
    mq = np.empty((EMBED_DIM, HD), dtype=f32)
    a2 = np.empty((128, 2, 64), dtype=f32)
    gzm = np.empty((64, HL, 64), dtype=f32)
    cqv = np.empty((HD,), dtype=f32)
    for hl in range(HL):
        h = g * HL + hl
        hs = slice(h * HEAD_DIM, (h + 1) * HEAD_DIM)
        L = K_land[:, hs].astype(np.float64)          # (64, 64)
        A2 = (2.0 / tau) * L.T                        # (d, k)
        a2[(hl % 2) * 64:(hl % 2) * 64 + 64, hl // 2, :] = A2.astype(f32)
        Ln2 = np.sum(L * L, axis=1)                   # |L_j|^2
        bq_h = bq[hs].astype(np.float64)
        cqv[hl * 64:(hl + 1) * 64] = ((-Ln2 + 2.0 * (bq_h @ L.T)) / tau).astype(f32)
        Wq_h = Wq[hs].astype(np.float64)              # (64, E)
        mq[:, hl * 64:(hl + 1) * 64] = (Wq_h.T @ A2).astype(f32)
        # landmark kernel + inverse
        D = np.maximum(Ln2[:, None] + Ln2[None, :] - 2.0 * (L @ L.T), 0.0)
        Wk_mat = np.exp(-D / tau) + REG * np.eye(NUM_LANDMARKS)
        Winv = np.linalg.inv(Wk_mat)
        gzm[:, hl, :] = (np.exp(-Ln2 / tau)[:, None] * Winv).astype(f32)  # G2^T
    # lhsT pair tiles: (e, p, 128, 128); pair p covers local heads 2p,2p+1
    mq_t = mq.reshape(NE, 128, 2, 128).transpose(0, 2, 1, 3)
    cq_t = cqv.reshape(2, 128).T.copy()              # (128, 2)
    return {
        "ntau": np.full((128, 1), -1.0 / tau, dtype=f32),
        "a2": a2.astype(bf16),
        "xt": xt_b,
        "wkv": np.ascontiguousarray(wkv).astype(bf16),
        "mq": np.ascontiguousarray(mq_t).astype(bf16),
        "cq": cq_t.astype(f32),
        "gz": gzm.astype(bf16),
    }


def kernel(query, Wq, bq, Wk, bk, Wv, bv, Wo, bo, temperature, landmark_idx):
    query = np.asarray(query)
    out_dtype = query.dtype
    f32 = np.float32
    x = query.astype(f32, copy=False)
    Wq, Wk, Wv, Wo = (np.asarray(w).astype(f32, copy=False) for w in (Wq, Wk, Wv, Wo))
    bq, bk, bv, bo = (np.asarray(v).astype(f32, copy=False) for v in (bq, bk, bv, bo))
    tau = float(np.asarray(temperature))
    idx = np.asarray(landmark_idx).astype(np.int64)

    general_ok = (
        x.shape == (BATCH, SEQ, EMBED_DIM)
        and Wq.shape == (EMBED_DIM, EMBED_DIM)
        and idx.shape == (NUM_LANDMARKS,)
        and not np.any(bk)
        and not np.any(bv)
        and tau > 0
    )
    if not general_ok:
        return _kernel_numpy(query, Wq, bq, Wk, bk, Wv, bv, Wo, bo,
                             temperature, landmark_idx)
    try:
        return _kernel_device(x, Wq, bq, Wk, bk, Wv, bv, Wo, bo, tau, idx,
                              out_dtype)
    except Exception:
        if DEBUG_RAISE:
            raise
        return _kernel_numpy(query, Wq, bq, Wk, bk, Wv, bv, Wo, bo,
                             temperature, landmark_idx)


def _kernel_device(x, Wq, bq, Wk, bk, Wv, bv, Wo, bo, tau, idx, out_dtype):
    import ml_dtypes
    from concourse import bass_utils
    bf16 = ml_dtypes.bfloat16

    nc = _build()

    f32 = np.float32
    xt = [np.ascontiguousarray(x[b].T).astype(bf16) for b in range(BATCH)]
    wos = []
    for g in range(HEAD_GROUPS):
        sl = slice(g * HD, (g + 1) * HD)
        WoT = np.ascontiguousarray(Wo[:, sl].T)      # (256, E)
        wos.append(np.ascontiguousarray(WoT.reshape(2, 128, EMBED_DIM)).astype(bf16))

    in_maps = []
    for b in range(BATCH):
        for g in range(HEAD_GROUPS):
            m = _prep_core_inputs(x[b], xt[b], Wq, bq, Wk, bk, Wv, bv, tau, idx, g)
            m["wo"] = wos[g]
            in_maps.append(m)

    global LAST_EXEC_NS, LAST_TRACE
    kw = {}
    if TRACE:
        _install_ntff_shim()
        kw = {"trace": True, "trace_cores": [0]}
    res = bass_utils.run_bass_kernel_spmd(nc, in_maps,
                                          core_ids=list(range(N_CORES)), **kw)
    if TRACE:
        LAST_EXEC_NS = res.exec_time_ns
        LAST_TRACE = res
    out = np.zeros((BATCH, SEQ, EMBED_DIM), dtype=f32)
    for b in range(BATCH):
        for g in range(HEAD_GROUPS):
            out[b] += res.results[b * HEAD_GROUPS + g]["op"]
        out[b] += bo
    return out.astype(out_dtype, copy=False)
